# revision 38
# baseline (speedup 1.0000x reference)
"""Trainium2 Bass kernel for nn_NodeNet: GNN message passing + 12-qubit TTN circuit.

Math: the reference's statevector circuit contracts exactly to per-node
Bloch-vector chains (every CNOT block keeps only its target wire; the
measurement is <Z_9>; the circuit is a tree so alive wires stay in
product states). Per node the whole circuit is ~60 scalar ops.

Sharding: E-parallel over the 8 cores. Core k owns edge columns
Ek = [1024k, 1024k+1024):
  bo_k^T[d,e] = sum_n X[n,d] Ro[n,e]      (local, contraction over nodes)
  beo_k[e,d]  = e[e] * bo_k[e,d]
  partial mi^T[d,n] = sum_{e in Ek} beo[e,d] RiT[e,n]
ReduceScatter sums the partials over cores and hands core k its own
128-node slice, which feeds the Bloch-chain circuit; per-core outputs
are concatenated on the host.

Precision: the relation matrices are 0/1-valued, so fp8e4 is exact and
quarters DMA bytes vs f32. X and beo are carried as bf16 high+low
splits packed side by side in the stationary operand, recovering
fp32-grade accuracy with the same matmul count; the split halves are
summed during PSUM eviction.

Circuit: computed in TRANSPOSED layout [rows, 128 nodes]. The linear
parts of each CNOT-block layer collapse into one small matmul whose
stationary matrix (theta-dependent) is packed on the host; only the
CNOT z-multiplies remain as elementwise vector ops. This replaces
~100 serialized [128,k] vector ops with 4 matmuls + ~12 vector ops.
"""

import ml_dtypes
import numpy as np

import bass_rust
import concourse.bass as bass
import concourse.mybir as mybir
import concourse.tile as tile
from concourse.bass_utils import run_bass_kernel_spmd
from concourse.masks import make_identity

F32 = mybir.dt.float32
BF16 = mybir.dt.bfloat16
FP8 = mybir.dt.float8e4
NP_FP8 = ml_dtypes.float8_e4m3
N_CORES = 8
N, E, D = 1024, 8192, 4
ES = E // N_CORES        # 1024 edge columns per core
P = 128                  # partitions / nodes per core
NCH = N // P             # 8 node chunks
ECH = ES // P            # 8 edge chunks per core
MW = 36                  # stationary width: high split at 0:4, low at 32:36
LO = 32                  # (PSUM partition reads must be 32-aligned)

_BLOCKS = [(0, 1, (0, 1)), (2, 3, (3, 2)), (4, 5, (4, 5)), (6, 7, (7, 6)),
           (8, 9, (8, 9)), (10, 11, (11, 10)), (1, 2, (1, 2)), (5, 6, (6, 5)),
           (9, 10, (10, 9)), (2, 5, (2, 5)), (5, 9, (5, 9))]

# Layer-A blocks in the column order that makes layer-B inputs contiguous:
# A-target wires [1, 6, 10, 2, 5, 9]; cols 0:3 feed B controls, 3:6 B targets.
A_BLOCKS = [0, 3, 5, 1, 2, 4]
B_BLOCKS = [6, 7, 8]

# Compute-engine operand partition windows must START at 0/32/64/96, so the
# sin tile S places its blocks at those bases:
#   S rows 0:8 sin(ar), 32:40 cos(ar), 40 ones (= sin(0 + pi/2)),
#   64:68 sin(X cols), 96:100 cos(X cols); everything else memset to 0.
SROWS = 104


def _srow(w):
    return w if w < 8 else 64 + (w - 8)


def _crow(w):
    return 32 + w if w < 8 else 96 + (w - 8)


ONES_S = 40

# aux tensor column layout (SROWS partitions x AUXW f32):
AUX_WA = 0            # [104, 50]
AUX_WB = 50           # [19, 41]
AUX_WC = 91           # [10, 65]
AUX_WD = 156          # [3, 1]
AUX_XT = 157          # [4, 128]
AUXW = AUX_XT + P

# ---------------------------------------------------------------------------
# Host-side circuit-constant preparation
# ---------------------------------------------------------------------------

_PAULI = np.array([
    [[0, 1], [1, 0]],
    [[0, -1j], [1j, 0]],
    [[1, 0], [0, -1]],
], dtype=np.complex128)


def _rot_so3(p):
    """SO(3) Bloch rotation of Rot(phi, theta, omega) = RZ(om) RY(th) RZ(phi)."""
    phi, th, om = float(p[0]), float(p[1]), float(p[2])
    c, s = np.cos(th / 2), np.sin(th / 2)
    U = np.array([
        [np.exp(-0.5j * (phi + om)) * c, -np.exp(0.5j * (phi - om)) * s],
        [np.exp(-0.5j * (phi - om)) * s, np.exp(0.5j * (phi + om)) * c],
    ])
    R = np.empty((3, 3))
    for i in range(3):
        for j in range(3):
            R[i, j] = 0.5 * np.real(
                np.trace(_PAULI[i] @ U @ _PAULI[j] @ U.conj().T))
    return R


def _pack_w(theta):
    """Stationary matrices for the transposed circuit matmuls.

    Each layer's linear map AND its CNOT z-multiplier vector are emitted by
    one matmul; the multiplier columns sit 32 partitions above the value
    columns so both PSUM reads start 32-aligned. Constant-1 entries come
    from the S ones-row (sin(pi/2)).

    psA [50,128]: rows 0:6 abx, 6:12 aby, 12:18 abz, 18 ones,
                  32:50 azx (1,1,..,az[t] at 38+t and 44+t).
    psB [41,128]: rows 0:9 {bbx j, bby 3+j, bbz 6+j}, 9 ones,
                  32:41 azxB (1 x3, az3[j] at 35+j and 38+j).
    psC [65,128]: rows 0:3 w5{x,y,z}, 32:35 {1, s9, s9}, 64 u.
    psD [1,128]: s10.
    """
    th = np.asarray(theta, np.float64)
    R = [_rot_so3(th[3 * k:3 * k + 3]) for k in range(23)]

    def rots(bidx):
        w1, w2, (c, tg) = _BLOCKS[bidx]
        k1, k2 = 2 * bidx, 2 * bidx + 1
        Rc = R[k1] if c == w1 else R[k2]
        Rt = R[k1] if tg == w1 else R[k2]
        return c, tg, Rc, Rt

    WA = np.zeros((SROWS, 50))
    WA[ONES_S, 18] = 1.0                  # vA ones row passthrough
    for r in range(6):
        WA[ONES_S, 32 + r] = 1.0          # azx over abx = identity
    for t, bidx in enumerate(A_BLOCKS):
        c, tg, Rc, Rt = rots(bidx)
        for i in range(3):
            WA[_srow(tg), 6 * i + t] = Rt[i, 0]
            WA[_crow(tg), 6 * i + t] = Rt[i, 2]
        for col in (32 + 6 + t, 32 + 12 + t):
            WA[_srow(c), col] = Rc[2, 0]
            WA[_crow(c), col] = Rc[2, 2]

    # vA rows 0:19 = {abx, aby*az, abz*az, ones}
    WB = np.zeros((19, 41))
    WB[18, 9] = 1.0                       # vB ones row passthrough
    for r in range(3):
        WB[18, 32 + r] = 1.0
    for j, bidx in enumerate(B_BLOCKS):
        c, tg, Rc, Rt = rots(bidx)
        acol, bcol = j, 3 + j
        for i in range(3):
            for comp in range(3):
                WB[6 * comp + bcol, 3 * i + j] = Rt[i, comp]
        for comp in range(3):
            for col in (32 + 3 + j, 32 + 6 + j):
                WB[6 * comp + acol, col] = Rc[2, comp]

    # vB rows 0:10 = {bbx, bby*az3, bbz*az3, ones}
    WC = np.zeros((10, 65))
    WC[9, 32] = 1.0                       # w5x multiplier = 1
    for comp in range(3):
        for i in range(3):
            WC[3 * comp + 1, i] = R[19][i, comp]   # w5 from b-col 1
        WC[3 * comp + 0, 33] = R[18][2, comp]      # s9 (scales w5y)
        WC[3 * comp + 0, 34] = R[18][2, comp]      # s9 (scales w5z)
        WC[3 * comp + 2, 64] = R[21][2, comp]      # u from b-col 2 (w9)

    # vC rows 0:3 = w5 * {1, s9, s9}
    WD = np.zeros((3, 1))
    WD[:, 0] = R[20][2, :]
    return (WA.astype(np.float32), WB.astype(np.float32),
            WC.astype(np.float32), WD.astype(np.float32))


# ---------------------------------------------------------------------------
# Walrus workaround: this build rejects >1 sync-wait per instruction
# ---------------------------------------------------------------------------


def _split_multi_waits(nc):
    for f in nc.m.functions:
        for bb in f.blocks:
            out = []
            for inst in bb.instructions:
                si = inst.sync_info
                if si is not None and si.on_wait and len(si.on_wait) > 1:
                    waits = list(si.on_wait)
                    for i, w in enumerate(waits[:-1]):
                        out.append(mybir.InstNoOp(
                            name=f"{inst.name}_wsplit{i}",
                            engine=inst.engine,
                            ins=[], outs=[],
                            sync_info=bass_rust.SyncInfo(
                                on_wait=[w], on_update=[]),
                        ))
                    inst.sync_info = bass_rust.SyncInfo(
                        on_wait=[waits[-1]], on_update=list(si.on_update))
                out.append(inst)
            bb.instructions = out


# ---------------------------------------------------------------------------
# Device kernel
# ---------------------------------------------------------------------------


def _build_nc():
    nc = bass.Bass("TRN2", target_bir_lowering=False, num_devices=N_CORES)

    ro_nat = nc.declare_dram_parameter("ro_nat", [N, ES], FP8, isOutput=False)
    ri_nat = nc.declare_dram_parameter("ri_nat", [N, ES], FP8, isOutput=False)
    rot_t = nc.declare_dram_parameter("rot_t", [ES, N], FP8, isOutput=False)
    rit_t = nc.declare_dram_parameter("rit_t", [ES, N], FP8, isOutput=False)
    xsp_d = nc.declare_dram_parameter("xsp", [P, NCH * MW], BF16,
                                      isOutput=False)
    ep_d = nc.declare_dram_parameter("eperm", [P, ECH], F32, isOutput=False)
    aux_d = nc.declare_dram_parameter("aux", [SROWS, AUXW], F32,
                                      isOutput=False)
    out = nc.declare_dram_parameter("out", [1, P], F32, isOutput=True)

    HPI = float(np.pi / 2)
    PI = float(np.pi)
    TWO_PI = float(2 * np.pi)
    MUL = mybir.AluOpType.mult
    ADD = mybir.AluOpType.add

    with tile.TileContext(nc) as tc:
        with (
            tc.tile_pool(name="big", bufs=1) as big,
            tc.tile_pool(name="small", bufs=1) as small,
            tc.tile_pool(name="work", bufs=1) as work,
            tc.tile_pool(name="acc", bufs=2, space="PSUM") as accp,
            tc.tile_pool(name="tbp", bufs=2, space="PSUM") as tbp,
            tc.tile_pool(name="dram", bufs=1, space="DRAM") as dram,
        ):
            # ---- small inputs: three DMAs ---------------------------------
            xsp_sb = small.tile([P, NCH * MW], BF16, name="xsp_sb")
            nc.sync.dma_start(xsp_sb[:], xsp_d[:])
            ep_sb = small.tile([P, ECH], F32, name="ep_sb")
            nc.sync.dma_start(ep_sb[:], ep_d[:])
            aux_sb = small.tile([SROWS, AUXW], F32, name="aux_sb")
            nc.sync.dma_start(aux_sb[:], aux_d[:])

            # preload the ACT Sin table set while DMAs stream
            warm = small.tile([P, 1], F32, name="warm")
            nc.vector.memset(warm[:], 0.0)
            nc.scalar.activation(warm[:], warm[:],
                                 mybir.ActivationFunctionType.Sin)

            ident = small.tile([P, P], F32, name="ident")
            make_identity(nc, ident)

            # ---- big matrix shards, one batched DMA per matrix ------------
            # chunk c of the [1024, 1024] matrix lands at cols [c*1024, ...)
            nat_sb = {}   # nat_sb[rel]: [128, NCH*ES] fp8, node chunks
            tt_sb = {}    # tt_sb[rel]: [128, ECH*N] fp8, edge chunks
            # rel o gates stage-1 start: its chunks alternate across both
            # queues so they land first at full aggregate bandwidth.
            to = big.tile([P, NCH * ES], FP8, name="nat_o", tag="nat_o")
            dsto = to.rearrange("p (c e) -> p c e", e=ES)
            svo = ro_nat.rearrange("(c p) e -> p c e", p=P)
            nc.sync.dma_start(dsto[:, 0:2, :], svo[:, 0:2, :])
            nc.gpsimd.dma_start(dsto[:, 2:4, :], svo[:, 2:4, :])
            nc.sync.dma_start(dsto[:, 4:6, :], svo[:, 4:6, :])
            nc.gpsimd.dma_start(dsto[:, 6:8, :], svo[:, 6:8, :])
            nat_sb["o"] = to

            # SDMA round-robin shares bandwidth across outstanding
            # descriptors, so keep the early in-flight set small: nat_i/tt_i
            # issue now (needed mid-phase); tt_o's DMA is issued later from
            # the scalar queue, after stage-2_o starts.
            ti = big.tile([P, NCH * ES], FP8, name="nat_i", tag="nat_i")
            nc.sync.dma_start(
                ti.rearrange("p (c e) -> p c e", e=ES),
                ri_nat.rearrange("(c p) e -> p c e", p=P))
            nat_sb["i"] = ti
            tt_sb["i"] = big.tile([P, ECH * N], FP8, name="tt_i", tag="tt_i")
            nc.gpsimd.dma_start(
                tt_sb["i"].rearrange("p (c n) -> p c n", n=N),
                rit_t.rearrange("(c p) n -> p c n", p=P))
            tt_sb["o"] = big.tile([P, ECH * N], FP8, name="tt_o", tag="tt_o")

            # ---- early dummy collective: absorbs the per-core launch
            # stagger while DMA/compute streams, so the real ReduceScatter's
            # rendezvous wait shrinks to compute-phase variance only.
            bar_in = dram.tile([8, 4], F32, name="bar_in")
            bar_out = dram.tile([1, 4], F32, name="bar_out")
            barz = small.tile([8, 4], F32, name="barz")
            nc.vector.memset(barz[:], 0.0)
            nc.sync.dma_start(bar_in[:], barz[:])
            nc.gpsimd.collective_compute(
                "ReduceScatter",
                mybir.AluOpType.add,
                replica_groups=[list(range(N_CORES))],
                ins=[bar_in.opt()],
                outs=[bar_out.opt()],
            )

            # ---- circuit sin tile; X rows computed pre-collective ---------
            # S rows: 0:8 sin(ar), 32:40 cos(ar), 40 ones, 64:68 sin(X),
            # 96:100 cos(X); all other rows zero.
            S = small.tile([SROWS, P], F32, name="S")
            nc.vector.memset(S[:], 0.0)

            def range_reduce_sin(dst, ang, nr, pfx):
                """dst[0:nr] = sin(reduce(ang[0:nr])); ang clobbered."""
                t_f = small.tile([nr, P], F32, name=f"{pfx}_tf")
                t_i = small.tile([nr, P], mybir.dt.int32, name=f"{pfx}_ti")
                t_r = small.tile([nr, P], F32, name=f"{pfx}_tr")
                nc.vector.tensor_scalar(
                    t_f[:], ang, float(1.0 / TWO_PI), None, MUL)
                nc.vector.tensor_copy(t_i[:], t_f[:])
                nc.vector.tensor_copy(t_r[:], t_i[:])
                nc.vector.scalar_tensor_tensor(
                    ang, t_r[:], -TWO_PI, ang, MUL, ADD)
                nc.vector.tensor_scalar(
                    ang, ang, PI, -PI,
                    mybir.AluOpType.min, mybir.AluOpType.max)
                nc.scalar.activation(dst, ang,
                                     mybir.ActivationFunctionType.Sin)

            # X angles at mx[0:4], +pi/2 at mx[32:36]; zeros elsewhere so the
            # sin lands at S rows 64:68 / 96:100 with 32-aligned accesses.
            mx = small.tile([36, P], F32, name="mx")
            nc.vector.memset(mx[:], 0.0)
            nc.vector.tensor_copy(mx[0:4], aux_sb[0:4, AUX_XT:AUX_XT + P])
            nc.vector.tensor_scalar(mx[32:36], mx[0:4], HPI, None, ADD)
            range_reduce_sin(S[64:100], mx[0:36], 36, "mx")

            # ---- stage 1: bo^T = [Xh|Xl]^T @ Ro, M=8 packed ---------------
            # ---- stage 2: transpose-back + e-scale + bf16 split -----------
            beo_sb = {}
            for rel in ("o", "i"):
                boT = work.tile([D, ES], F32, name=f"boT_{rel}",
                                tag=f"boT_{rel}")
                for h in range(2):
                    ps = accp.tile([MW, 512], F32, name=f"boT_ps_{rel}{h}",
                                   tag="acc")
                    for c in range(NCH):
                        nc.tensor.matmul(
                            ps[:],
                            xsp_sb[:, c * MW:(c + 1) * MW],
                            nat_sb[rel][:, c * ES + h * 512:
                                        c * ES + (h + 1) * 512],
                            start=(c == 0), stop=(c == NCH - 1))
                    lo_t = small.tile([D, 512], F32, name=f"lo_b{rel}{h}",
                                      tag="lo_t", bufs=2)
                    nc.scalar.copy(lo_t[:], ps[LO:LO + 4, :])
                    nc.vector.tensor_add(
                        boT[:, h * 512:(h + 1) * 512], ps[0:4, :], lo_t[:])
                # per-chunk transpose + e-scale + bf16 hi/lo split so stage-3
                # chunk c can start as soon as its slice is split
                bhl = work.tile([P, ECH * 2 * D], BF16, name=f"bhl_{rel}",
                                tag=f"bhl_{rel}")
                hl4 = bhl.rearrange("p (c m) -> p c m", m=2 * D)
                for c in range(ECH):
                    tb = tbp.tile([P, D], F32, name=f"tb_{rel}{c}", tag="tb")
                    nc.tensor.transpose(
                        tb[:], boT[:, c * P:(c + 1) * P], ident[0:D, 0:D])
                    bc = small.tile([P, D], F32, name=f"bc_{rel}{c}",
                                    tag="bc", bufs=2)
                    rc = small.tile([P, D], F32, name=f"rc_{rel}{c}",
                                    tag="rc", bufs=2)
                    nc.vector.tensor_scalar(
                        bc[:], tb[:], ep_sb[:, c:c + 1], None, MUL)
                    nc.vector.tensor_copy(hl4[:, c, 0:D], bc[:])
                    nc.vector.scalar_tensor_tensor(
                        rc[:], hl4[:, c, 0:D], -1.0, bc[:], MUL, ADD)
                    nc.vector.tensor_copy(hl4[:, c, D:2 * D], rc[:])
                beo_sb[rel] = bhl
                if rel == "o":
                    # tt_o is last-needed: issue its DMA only now (from the
                    # scalar queue) so it doesn't steal early bandwidth
                    nc.scalar.dma_start(
                        tt_sb["o"].rearrange("p (c n) -> p c n", n=N),
                        rot_t.rearrange("(c p) n -> p c n", p=P))

            # ---- stage 3: partial mi^T = [beo_h|beo_l]^T @ RiT, M=8 -------
            # mi pairs beo (from Ro) with RiT; mo pairs bei with RoT.
            # ar rows: 0:8 hi [mi0..3, mo0..3], 8:16 lo; the hi+lo add and
            # the cross-core reduction both happen after the AllToAll, so
            # the PSUM halves are DMAd straight into the collective input.
            ar_in = dram.tile([NCH, 16, P], F32, name="ar_in")
            ar_out = dram.tile([16, P], F32, name="ar_out")
            for ri, (rel_b, rel_t) in enumerate((("o", "i"), ("i", "o"))):
                for h in range(2):
                    ps = accp.tile([2 * D, 512], F32, name=f"miT_ps_{ri}{h}",
                                   tag="acc3")
                    for c in range(ECH):
                        nc.tensor.matmul(
                            ps[:],
                            beo_sb[rel_b][:, 2 * D * c:2 * D * (c + 1)],
                            tt_sb[rel_t][:, c * N + h * 512:
                                         c * N + (h + 1) * 512],
                            start=(c == 0), stop=(c == ECH - 1))
                    mhl = small.tile([2 * D, 512], F32, name=f"mhl_{ri}{h}",
                                     tag="mhl", bufs=2)
                    nc.vector.tensor_copy(mhl[:], ps[:])
                    # node n = h*512 + col -> chunks 4h..4h+3; psum row
                    # g*4+d -> ar row g*8 + 4*ri + d
                    nc.sync.dma_start(
                        ar_in[4 * h:4 * h + 4, 4 * ri:4 * ri + 4]
                        .rearrange("c r p -> r c p"),
                        mhl[0:D].rearrange("d (c p) -> d c p", p=P))
                    nc.sync.dma_start(
                        ar_in[4 * h:4 * h + 4, 8 + 4 * ri:8 + 4 * ri + 4]
                        .rearrange("c r p -> r c p"),
                        mhl[D:2 * D].rearrange("d (c p) -> d c p", p=P))

            nc.gpsimd.collective_compute(
                "ReduceScatter",
                mybir.AluOpType.add,
                replica_groups=[list(range(N_CORES))],
                ins=[ar_in.opt()],
                outs=[ar_out.opt()],
            )

            # ---- circuit: transposed layout [rows, 128 nodes] -------------
            # reduced hi rows land at prs[0:8], lo rows at prs[32:40]; the
            # hi+lo add produces the ar angles at mr[0:8], +pi/2 at
            # mr[32:40]; row 40 = pi/2 so S row 40 becomes the ones row.
            prh = small.tile([8, P], F32, name="prh")
            prl = small.tile([8, P], F32, name="prl")
            nc.sync.dma_start(prh[:], ar_out[0:8])
            nc.gpsimd.dma_start(prl[:], ar_out[8:16])
            mr = small.tile([41, P], F32, name="mr")
            nc.vector.memset(mr[:], 0.0)
            nc.vector.tensor_add(mr[0:8], prh[:], prl[:])
            nc.vector.tensor_scalar(mr[32:41], mr[0:9], HPI, None, ADD)
            range_reduce_sin(S[0:41], mr[0:41], 41, "mr")

            # layer A: one matmul emits values AND the CNOT z-multipliers
            psA = tbp.tile([50, P], F32, name="psA", tag="cc")
            nc.tensor.matmul(psA[:], aux_sb[:, AUX_WA:AUX_WA + 50], S[:],
                             start=True, stop=True)
            vA = small.tile([19, P], F32, name="vA")
            vAZ = small.tile([18, P], F32, name="vAZ")
            nc.vector.tensor_copy(vA[:], psA[0:19])
            nc.scalar.copy(vAZ[:], psA[32:50])
            nc.vector.tensor_tensor(vA[0:18], vA[0:18], vAZ[:], MUL)

            # layer B
            psB = tbp.tile([41, P], F32, name="psB", tag="cc")
            nc.tensor.matmul(psB[:], aux_sb[0:19, AUX_WB:AUX_WB + 41],
                             vA[:], start=True, stop=True)
            vB = small.tile([10, P], F32, name="vB")
            vBZ = small.tile([9, P], F32, name="vBZ")
            nc.vector.tensor_copy(vB[:], psB[0:10])
            nc.scalar.copy(vBZ[:], psB[32:41])
            nc.vector.tensor_tensor(vB[0:9], vB[0:9], vBZ[:], MUL)

            # layer C: block 9 (w2 -> w5), then block 10 (w5 -> w9)
            psC = tbp.tile([65, P], F32, name="psC", tag="cc")
            nc.tensor.matmul(psC[:], aux_sb[0:10, AUX_WC:AUX_WC + 65],
                             vB[:], start=True, stop=True)
            vC = small.tile([3, P], F32, name="vC")
            vCZ = small.tile([3, P], F32, name="vCZ")
            uT = small.tile([1, P], F32, name="uT")
            nc.vector.tensor_copy(vC[:], psC[0:3])
            nc.scalar.copy(vCZ[:], psC[32:35])
            nc.scalar.copy(uT[:], psC[64:65])
            nc.vector.tensor_tensor(vC[:], vC[:], vCZ[:], MUL)

            psD = tbp.tile([1, P], F32, name="psD", tag="cc")
            nc.tensor.matmul(psD[:], aux_sb[0:3, AUX_WD:AUX_WD + 1],
                             vC[:], start=True, stop=True)
            zf = small.tile([1, P], F32, name="zf")
            nc.vector.tensor_tensor(zf[:], psD[0:1], uT[:], MUL)
            res = small.tile([1, P], F32, name="res")
            nc.vector.tensor_scalar(res[:], zf[:], -PI, PI, MUL, ADD)
            nc.sync.dma_start(out[:], res[:])

    return nc


_NC_CACHE = {}
_RUN_KWARGS = {}      # test harness can set e.g. {"trace": True}
_LAST_RESULTS = []    # BassKernelResults of the most recent run


def _get_nc():
    if "nc" not in _NC_CACHE:
        nc = _build_nc()
        _split_multi_waits(nc)
        _NC_CACHE["nc"] = nc
    return _NC_CACHE["nc"]


def _host_prep_x(X):
    """xsp[p, c*MW + {0:4,LO:LO+4}] = {high,low} split of X[c*128+p, :]."""
    bf = ml_dtypes.bfloat16
    xh = X.astype(bf).astype(np.float32)
    xl = X - xh
    xsp = np.zeros((P, NCH, MW), np.float32)
    xsp[:, :, 0:D] = xh.reshape(NCH, P, D).transpose(1, 0, 2)
    xsp[:, :, LO:LO + D] = xl.reshape(NCH, P, D).transpose(1, 0, 2)
    return np.ascontiguousarray(xsp.reshape(P, NCH * MW).astype(bf))


def kernel(X, e, Ri, Ro, theta):
    X = np.ascontiguousarray(np.asarray(X, np.float32))
    e = np.ascontiguousarray(np.asarray(e, np.float32))
    Ri = np.asarray(Ri, np.float32)
    Ro = np.asarray(Ro, np.float32)
    theta = np.asarray(theta, np.float32)

    xsp = _host_prep_x(X)
    WA, WB, WC, WD = _pack_w(theta)
    aux0 = np.zeros((SROWS, AUXW), np.float32)
    aux0[0:SROWS, AUX_WA:AUX_WA + 50] = WA
    aux0[0:19, AUX_WB:AUX_WB + 41] = WB
    aux0[0:10, AUX_WC:AUX_WC + 65] = WC
    aux0[0:3, AUX_WD:AUX_WD + 1] = WD

    in_maps = []
    for k in range(N_CORES):
        ek = slice(k * ES, (k + 1) * ES)
        aux = aux0.copy()
        aux[0:4, AUX_XT:AUX_XT + P] = X[k * P:(k + 1) * P, :].T
        in_maps.append({
            "ro_nat": np.ascontiguousarray(Ro[:, ek].astype(NP_FP8)),
            "ri_nat": np.ascontiguousarray(Ri[:, ek].astype(NP_FP8)),
            "rot_t": np.ascontiguousarray(Ro[:, ek].T.astype(NP_FP8)),
            "rit_t": np.ascontiguousarray(Ri[:, ek].T.astype(NP_FP8)),
            "xsp": xsp,
            "eperm": np.ascontiguousarray(e[ek].reshape(ECH, P).T),
            "aux": np.ascontiguousarray(aux),
        })

    nc = _get_nc()
    res = run_bass_kernel_spmd(nc, in_maps, core_ids=list(range(N_CORES)),
                               **_RUN_KWARGS)
    _LAST_RESULTS.clear()
    _LAST_RESULTS.append(res)
    return np.concatenate(
        [res.results[k]["out"].reshape(-1) for k in range(N_CORES)]
    ).astype(np.float32)
